# revision 1
# baseline (speedup 1.0000x reference)
"""NNUE forward kernel for Trainium2, 8-core SPMD, batch-sharded.

Reference computation (B=4096, I=40960, H=256):
    h_p = clip(x_p @ W_p.T + b_p, 0, 1)   for p in {1,2}
    out = concat(h1, h2) @ v + b2         -> (B,)

Sharding: data-parallel over batch. Each of the 8 cores gets 512 batch
rows of x1/x2 and a replicated copy of the (transposed, bf16) weights.

Dense baseline: per core, stream xT (40960x512) and WT (40960x256) tiles
of 128 contraction rows; accumulate h.T = W @ x.T in PSUM as
[128h x 512b] tiles (2 h-tiles per perspective); bias+clip on VectorE;
second layer as a [1x512] PE contraction over the 4 h-chunks.
"""

import numpy as np
import ml_dtypes

import concourse.bass as bass
import concourse.mybir as mybir
from concourse import bacc
from concourse.tile import TileContext
from concourse.bass_utils import run_bass_kernel_spmd

BATCH = 4096
INPUT_SIZE = 40960
HIDDEN = 256
N_CORES = 8
B_CORE = BATCH // N_CORES  # 512
K_TILES = INPUT_SIZE // 128  # 320

BF16 = mybir.dt.bfloat16
F32 = mybir.dt.float32

_NC_CACHE = {}


def _build_dense():
    nc = bacc.Bacc("TRN2", target_bir_lowering=False, debug=False)

    xt = nc.dram_tensor("xt", [2, INPUT_SIZE, B_CORE], BF16, kind="ExternalInput")
    wt = nc.dram_tensor("wt", [2, INPUT_SIZE, HIDDEN], BF16, kind="ExternalInput")
    b1 = nc.dram_tensor("b1", [2, HIDDEN], F32, kind="ExternalInput")
    v = nc.dram_tensor("v", [4, 128], F32, kind="ExternalInput")
    b2 = nc.dram_tensor("b2", [1, 1], F32, kind="ExternalInput")
    out = nc.dram_tensor("out", [1, B_CORE], F32, kind="ExternalOutput")

    with TileContext(nc) as tc:
        with (
            tc.tile_pool(name="io", bufs=4) as io,
            tc.tile_pool(name="consts", bufs=1) as consts,
            tc.tile_pool(name="psum", bufs=1, space="PSUM") as pp,
            tc.tile_pool(name="ep", bufs=2) as ep,
        ):
            # Constants
            v_tile = consts.tile([128, 4], F32, tag="v")
            for i in range(4):
                nc.sync.dma_start(out=v_tile[:, i : i + 1], in_=v[i, :])
            b1_tiles = []
            for p in range(2):
                for hi in range(2):
                    t = consts.tile([128, 1], F32, tag=f"b1_{p}_{hi}", name=f"b1_{p}_{hi}")
                    nc.sync.dma_start(
                        out=t, in_=b1[p, hi * 128 : (hi + 1) * 128]
                    )
                    b1_tiles.append(t)
            b2_tile = consts.tile([1, 1], F32, tag="b2")
            nc.sync.dma_start(out=b2_tile, in_=b2[:, :])

            # Four persistent PSUM accumulators: (p, hi) -> [128h x 512b]
            psums = [pp.tile([128, B_CORE], F32, tag=f"acc{i}", name=f"acc{i}") for i in range(4)]

            for p in range(2):
                for k in range(K_TILES):
                    xt_t = io.tile([128, B_CORE], BF16, tag="xt")
                    wt_t = io.tile([128, HIDDEN], BF16, tag="wt")
                    nc.sync.dma_start(
                        out=xt_t, in_=xt[p, k * 128 : (k + 1) * 128, :]
                    )
                    nc.sync.dma_start(
                        out=wt_t, in_=wt[p, k * 128 : (k + 1) * 128, :]
                    )
                    for hi in range(2):
                        nc.tensor.matmul(
                            psums[p * 2 + hi],
                            lhsT=wt_t[:, hi * 128 : (hi + 1) * 128],
                            rhs=xt_t,
                            start=(k == 0),
                            stop=(k == K_TILES - 1),
                        )

            # Epilogue: bias + clip, then second layer.
            ps_out = pp.tile([1, B_CORE], F32, tag="out")
            for i in range(4):
                cl = ep.tile([128, B_CORE], F32, tag="cl")
                nc.vector.tensor_scalar_add(cl, psums[i], b1_tiles[i])
                nc.vector.tensor_scalar(
                    cl, cl, 0.0, 1.0, op0=mybir.AluOpType.max, op1=mybir.AluOpType.min
                )
                nc.tensor.matmul(
                    ps_out,
                    lhsT=v_tile[:, i : i + 1],
                    rhs=cl,
                    start=(i == 0),
                    stop=(i == 3),
                )
            sb_out = ep.tile([1, B_CORE], F32, tag="sbout")
            nc.vector.tensor_scalar_add(sb_out, ps_out, b2_tile)
            nc.sync.dma_start(out=out[:, :], in_=sb_out)

    nc.compile()
    return nc


def _run(x1, x2, l1_weights, l1_biases, l2_weight, l2_bias, trace=False):
    if "dense" not in _NC_CACHE:
        _NC_CACHE["dense"] = _build_dense()
    nc = _NC_CACHE["dense"]

    bf16 = ml_dtypes.bfloat16
    # Host-side prep: transposed bf16 weights (shared across cores).
    wt_np = np.ascontiguousarray(
        l1_weights.astype(np.float32).transpose(0, 2, 1)
    ).astype(bf16)  # [2, I, H]
    b1_np = l1_biases.astype(np.float32)  # [2, 256]
    v_np = np.ascontiguousarray(
        l2_weight.astype(np.float32).reshape(4, 128)
    )
    b2_np = l2_bias.astype(np.float32).reshape(1, 1)

    in_maps = []
    for c in range(N_CORES):
        sl = slice(c * B_CORE, (c + 1) * B_CORE)
        xt_np = np.empty((2, INPUT_SIZE, B_CORE), dtype=bf16)
        xt_np[0] = x1[sl].T.astype(bf16)
        xt_np[1] = x2[sl].T.astype(bf16)
        in_maps.append(
            {
                "xt": xt_np,
                "wt": wt_np,
                "b1": b1_np,
                "v": v_np,
                "b2": b2_np,
            }
        )

    res = run_bass_kernel_spmd(
        nc, in_maps, core_ids=list(range(N_CORES)), trace=trace
    )
    out = np.concatenate(
        [res.results[c]["out"].reshape(B_CORE) for c in range(N_CORES)]
    )
    return out.astype(np.float32), res


def kernel(**inputs):
    out, _ = _run(**inputs)
    return out


def kernel_profiled(**inputs):
    _, res = _run(**inputs, trace=True)
    return res



# revision 4
# speedup vs baseline: 3.2806x; 3.2806x over previous
"""NNUE forward kernel for Trainium2, 8-core SPMD, batch-sharded,
sparsity-exploiting (embedding-gather formulation).

Reference computation (B=4096, I=40960, H=256):
    h_p = clip(x_p @ W_p.T + b_p, 0, 1)   for p in {1,2}
    out = concat(h1, h2) @ v + b2         -> (B,)

x_p rows are sparse binary (~30 active features of 40960), so
x_p @ W_p.T is an embedding-sum: h[b] = sum_{active f} W_p.T[f, :].

Per core (512 batch rows): for each 128-row tile and perspective,
DMA-gather the active rows of the bf16 table W_p.T (split into two
20480-row halves so indices fit int16), then reduce the gathered slots
into per-row sums on the PE with a per-tile 0/1 selector matrix S
(fp8): h_tile[128, 256] = S.T @ gathered. Epilogue (bias, clip, dot
with v, + b2) runs on the Vector engine. No collectives (pure data
parallel; batch-sharded).
"""

import numpy as np
import ml_dtypes

import concourse.bass as bass
import concourse.mybir as mybir
from concourse import bacc
from concourse.tile import TileContext
from concourse.bass_utils import run_bass_kernel_spmd

BATCH = 4096
INPUT_SIZE = 40960
HIDDEN = 256
N_CORES = 8
B_CORE = BATCH // N_CORES  # 512
N_TILES = B_CORE // 128  # 4
HALF = INPUT_SIZE // 2  # 20480 rows per table half (int16 index range)

BF16 = mybir.dt.bfloat16
F32 = mybir.dt.float32
F8 = mybir.dt.float8e4
I16 = mybir.dt.int16

NP_BF16 = ml_dtypes.bfloat16
NP_F8 = mybir.dt.np(F8)

_NC_CACHE = {}


def _build(nh):
    """nh: padded gather count per (128-row tile, perspective, half)."""
    nhc = nh // 16  # idx columns (16-partition wrap)
    ncol = nh // 128  # gather output columns per half
    cc = 2 * ncol  # selector columns per (tile, persp)

    nc = bacc.Bacc("TRN2", target_bir_lowering=False, debug=False)

    tbl = [
        [
            nc.dram_tensor(f"t{p}{h}", [HALF + 1, HIDDEN], BF16, kind="ExternalInput")
            for h in range(2)
        ]
        for p in range(2)
    ]
    idxd = nc.dram_tensor("idx", [16, 128, nhc], I16, kind="ExternalInput")
    smatd = nc.dram_tensor("smat", [8, 128, cc * 128], F8, kind="ExternalInput")
    biasd = nc.dram_tensor("bias", [128, 2, HIDDEN], F32, kind="ExternalInput")
    vd = nc.dram_tensor("v", [128, 2, HIDDEN], F32, kind="ExternalInput")
    b2d = nc.dram_tensor("b2", [128, 1], F32, kind="ExternalInput")
    outd = nc.dram_tensor("out", [128, N_TILES], F32, kind="ExternalOutput")

    with TileContext(nc) as tc:
        with (
            tc.tile_pool(name="consts", bufs=1) as consts,
            tc.tile_pool(name="sp", bufs=2) as sp,
            tc.tile_pool(name="g0p", bufs=2) as g0p,
            tc.tile_pool(name="g1p", bufs=2) as g1p,
            tc.tile_pool(name="psum", bufs=2, space="PSUM") as pp,
            tc.tile_pool(name="ep", bufs=2) as ep,
        ):
            idxt = consts.tile([128, 16, nhc], I16, tag="idx")
            for g in range(16):
                nc.sync.dma_start(out=idxt[:, g, :], in_=idxd[g, :, :])
            bias_t = consts.tile([128, 2, HIDDEN], F32, tag="bias")
            nc.sync.dma_start(out=bias_t, in_=biasd[:, :, :])
            v_t = consts.tile([128, 2, HIDDEN], F32, tag="v")
            nc.sync.dma_start(out=v_t, in_=vd[:, :, :])
            b2_t = consts.tile([128, 1], F32, tag="b2")
            nc.sync.dma_start(out=b2_t, in_=b2d[:, :])
            outst = consts.tile([128, N_TILES], F32, tag="outst")

            acc0 = None
            for i in range(2 * N_TILES):
                t, p = i // 2, i % 2
                s_t = sp.tile([128, cc * 128], F8, tag="smat")
                nc.sync.dma_start(out=s_t, in_=smatd[i, :, :])
                g0 = g0p.tile([128, ncol, HIDDEN], BF16, tag="g0")
                g1 = g1p.tile([128, ncol, HIDDEN], BF16, tag="g1")
                # HW SWDGE limit: >1024 idxs in one dma_gather wedges the
                # exec unit — chunk to <=1024 (8 output columns) per call.
                for h, gt in ((0, g0), (1, g1)):
                    for k0 in range(0, nh, 1024):
                        k1 = min(k0 + 1024, nh)
                        nc.gpsimd.dma_gather(
                            gt[:, k0 // 128 : k1 // 128, :],
                            tbl[p][h][:, :],
                            idxt[:, 2 * i + h, k0 // 16 : k1 // 16],
                            k1 - k0,
                            k1 - k0,
                            HIDDEN,
                        )
                psum = pp.tile([128, HIDDEN], F32, tag="psum")
                for j in range(cc):
                    g = g0 if j < ncol else g1
                    nc.tensor.matmul(
                        psum,
                        lhsT=s_t[:, j * 128 : (j + 1) * 128],
                        rhs=g[:, j % ncol, :],
                        start=(j == 0),
                        stop=(j == cc - 1),
                    )
                cl = ep.tile([128, HIDDEN], F32, tag="cl")
                nc.vector.tensor_tensor(cl, psum, bias_t[:, p, :], op=mybir.AluOpType.add)
                nc.vector.tensor_scalar(
                    cl, cl, 0.0, 1.0, op0=mybir.AluOpType.max, op1=mybir.AluOpType.min
                )
                prod = ep.tile([128, HIDDEN], F32, tag="prod")
                nc.vector.tensor_tensor(
                    prod, cl, v_t[:, p, :], op=mybir.AluOpType.mult
                )
                if p == 0:
                    acc0 = ep.tile([128, 1], F32, tag="acc0")
                    nc.vector.tensor_reduce(
                        acc0, prod, axis=mybir.AxisListType.X, op=mybir.AluOpType.add
                    )
                else:
                    acc1 = ep.tile([128, 1], F32, tag="acc1")
                    nc.vector.tensor_reduce(
                        acc1, prod, axis=mybir.AxisListType.X, op=mybir.AluOpType.add
                    )
                    # out[:, t] = (acc0 + b2) + acc1
                    nc.vector.scalar_tensor_tensor(
                        outst[:, t : t + 1],
                        acc0,
                        b2_t,
                        acc1,
                        op0=mybir.AluOpType.add,
                        op1=mybir.AluOpType.add,
                    )
            nc.sync.dma_start(out=outd[:, :], in_=outst)

    nc.compile()
    return nc


def _prep(x1, x2, l1_weights, l1_biases, l2_weight, l2_bias):
    """Host-side: tables, per-core index lists + selector matrices."""
    wt = np.ascontiguousarray(
        l1_weights.astype(np.float32).transpose(0, 2, 1)
    )  # [2, I, H]
    tabs = {}
    for p in range(2):
        for h in range(2):
            tt = np.zeros((HALF + 1, HIDDEN), dtype=NP_BF16)
            tt[:HALF] = wt[p, h * HALF : (h + 1) * HALF].astype(NP_BF16)
            tabs[f"t{p}{h}"] = tt

    bias_full = np.ascontiguousarray(
        np.broadcast_to(
            l1_biases.astype(np.float32)[None, :, :], (128, 2, HIDDEN)
        )
    )
    v_full = np.ascontiguousarray(
        np.broadcast_to(
            l2_weight.astype(np.float32).reshape(1, 2, HIDDEN), (128, 2, HIDDEN)
        )
    )
    b2_full = np.full((128, 1), float(np.asarray(l2_bias).reshape(-1)[0]), np.float32)

    xs = [np.asarray(x1), np.asarray(x2)]
    # (core, tile, persp, half) -> (rows, feats) of active entries
    active = []
    max_n = 0
    for c in range(N_CORES):
        per_core = []
        for i in range(2 * N_TILES):
            t, p = i // 2, i % 2
            blk = xs[p][c * B_CORE + t * 128 : c * B_CORE + (t + 1) * 128]
            r_all, f_all = np.nonzero(blk)
            for h in range(2):
                sel = (f_all >= h * HALF) & (f_all < (h + 1) * HALF)
                r, f = r_all[sel], f_all[sel] - h * HALF
                per_core.append((r.astype(np.int32), f.astype(np.int32)))
                max_n = max(max_n, len(r))
        active.append(per_core)

    nh = max(2048, -(-max_n // 128) * 128)
    nhc = nh // 16
    ncol = nh // 128
    cc = 2 * ncol

    in_maps = []
    for c in range(N_CORES):
        idx_arr = np.empty((16, 128, nhc), np.int16)
        smat = np.zeros((8, 128, cc * 128), NP_F8)
        for i in range(2 * N_TILES):
            for h in range(2):
                r, f = active[c][i * 2 + h]
                n = len(r)
                idxv = np.full(nh, HALF, np.int16)
                idxv[:n] = f
                idx_arr[i * 2 + h] = np.tile(idxv.reshape(nhc, 16).T, (8, 1))
                j = np.arange(n)
                smat[i, j % 128, (j // 128 + h * ncol) * 128 + r] = 1.0
        in_map = dict(tabs)
        in_map.update(
            idx=idx_arr,
            smat=smat,
            bias=bias_full,
            v=v_full,
            b2=b2_full,
        )
        in_maps.append(in_map)
    return nh, in_maps


def _run(x1, x2, l1_weights, l1_biases, l2_weight, l2_bias, trace=False):
    nh, in_maps = _prep(x1, x2, l1_weights, l1_biases, l2_weight, l2_bias)
    if nh not in _NC_CACHE:
        _NC_CACHE[nh] = _build(nh)
    nc = _NC_CACHE[nh]

    res = run_bass_kernel_spmd(
        nc, in_maps, core_ids=list(range(N_CORES)), trace=trace
    )
    out = np.concatenate(
        [
            np.ascontiguousarray(res.results[c]["out"].T).reshape(B_CORE)
            for c in range(N_CORES)
        ]
    )
    return out.astype(np.float32), res


def kernel(**inputs):
    out, _ = _run(**inputs)
    return out


def kernel_profiled(**inputs):
    _, res = _run(**inputs, trace=True)
    return res


# revision 6
# speedup vs baseline: 7.6694x; 2.3378x over previous
"""NNUE forward kernel for Trainium2, 8-core SPMD, batch-sharded,
sparsity-exploiting (embedding-gather formulation).

Reference computation (B=4096, I=40960, H=256):
    h_p = clip(x_p @ W_p.T + b_p, 0, 1)   for p in {1,2}
    out = concat(h1, h2) @ v + b2         -> (B,)

x_p rows are sparse binary (~30 active features of 40960), so
x_p @ W_p.T is an embedding-sum: h[b] = sum_{active f} W_p.T[f, :].

Per core (512 batch rows): for each 128-row tile and perspective,
DMA-gather the active rows of the bf16 table W_p.T (split into two
20480-row halves so indices fit int16), then reduce the gathered slots
into per-row sums on the PE with a per-tile 0/1 selector matrix S
(fp8): h_tile[128, 256] = S.T @ gathered. Epilogue (bias, clip, dot
with v, + b2) runs on the Vector engine. No collectives (pure data
parallel; batch-sharded).
"""

import numpy as np
import ml_dtypes

import concourse.bass as bass
import concourse.mybir as mybir
from concourse import bacc
from concourse.tile import TileContext
from concourse.bass_utils import run_bass_kernel_spmd

BATCH = 4096
INPUT_SIZE = 40960
HIDDEN = 256
N_CORES = 8
B_CORE = BATCH // N_CORES  # 512
N_TILES = B_CORE // 128  # 4
HALF = INPUT_SIZE // 2  # 20480 rows per table half (int16 index range)

BF16 = mybir.dt.bfloat16
F32 = mybir.dt.float32
F8 = mybir.dt.float8e4
I16 = mybir.dt.int16

NP_BF16 = ml_dtypes.bfloat16
NP_F8 = mybir.dt.np(F8)

_NC_CACHE = {}


def _build(nh):
    """nh: padded gather count per (128-row tile, perspective, half)."""
    nhc = nh // 16  # idx columns (16-partition wrap)
    ncol = nh // 128  # gather output columns per half
    cc = 2 * ncol  # selector columns per (tile, persp)

    nc = bacc.Bacc(
        "TRN2", target_bir_lowering=False, debug=False, num_swdge_queues=4
    )

    tbl = [
        [
            nc.dram_tensor(f"t{p}{h}", [HALF + 1, HIDDEN], BF16, kind="ExternalInput")
            for h in range(2)
        ]
        for p in range(2)
    ]
    idxd = nc.dram_tensor("idx", [16, 128, nhc], I16, kind="ExternalInput")
    smatd = nc.dram_tensor("smat", [8, 128, cc * 128], F8, kind="ExternalInput")
    biasd = nc.dram_tensor("bias", [128, 2, HIDDEN], F32, kind="ExternalInput")
    vd = nc.dram_tensor("v", [128, 2, HIDDEN], F32, kind="ExternalInput")
    b2d = nc.dram_tensor("b2", [128, 1], F32, kind="ExternalInput")
    outd = nc.dram_tensor("out", [128, N_TILES], F32, kind="ExternalOutput")

    with TileContext(nc) as tc:
        with (
            tc.tile_pool(name="consts", bufs=1) as consts,
            tc.tile_pool(name="sp", bufs=2) as sp,
            tc.tile_pool(name="g0p", bufs=2) as g0p,
            tc.tile_pool(name="g1p", bufs=2) as g1p,
            tc.tile_pool(name="psum", bufs=2, space="PSUM") as pp,
            tc.tile_pool(name="ep", bufs=2) as ep,
        ):
            idxt = consts.tile([128, 16, nhc], I16, tag="idx")
            for g in range(16):
                nc.sync.dma_start(out=idxt[:, g, :], in_=idxd[g, :, :])
            bias_t = consts.tile([128, 2, HIDDEN], F32, tag="bias")
            nc.sync.dma_start(out=bias_t, in_=biasd[:, :, :])
            v_t = consts.tile([128, 2, HIDDEN], F32, tag="v")
            nc.sync.dma_start(out=v_t, in_=vd[:, :, :])
            b2_t = consts.tile([128, 1], F32, tag="b2")
            nc.sync.dma_start(out=b2_t, in_=b2d[:, :])
            outst = consts.tile([128, N_TILES], F32, tag="outst")

            acc0 = None
            for i in range(2 * N_TILES):
                t, p = i // 2, i % 2
                s_t = sp.tile([128, cc * 128], F8, tag="smat")
                nc.sync.dma_start(out=s_t, in_=smatd[i, :, :])
                g0 = g0p.tile([128, ncol, HIDDEN], BF16, tag="g0")
                g1 = g1p.tile([128, ncol, HIDDEN], BF16, tag="g1")
                # HW SWDGE limit: >1024 idxs in one dma_gather wedges the
                # exec unit — chunk to <=1024 (8 output columns) per call.
                q = 0
                for h, gt in ((0, g0), (1, g1)):
                    for k0 in range(0, nh, 1024):
                        k1 = min(k0 + 1024, nh)
                        nc.gpsimd.dma_gather(
                            gt[:, k0 // 128 : k1 // 128, :],
                            tbl[p][h][:, :],
                            idxt[:, 2 * i + h, k0 // 16 : k1 // 16],
                            k1 - k0,
                            k1 - k0,
                            HIDDEN,
                            queue_num=q % 4,
                        )
                        q += 1
                psum = pp.tile([128, HIDDEN], F32, tag="psum")
                for j in range(cc):
                    g = g0 if j < ncol else g1
                    nc.tensor.matmul(
                        psum,
                        lhsT=s_t[:, j * 128 : (j + 1) * 128],
                        rhs=g[:, j % ncol, :],
                        start=(j == 0),
                        stop=(j == cc - 1),
                    )
                cl = ep.tile([128, HIDDEN], F32, tag="cl")
                nc.vector.tensor_tensor(cl, psum, bias_t[:, p, :], op=mybir.AluOpType.add)
                nc.vector.tensor_scalar(
                    cl, cl, 0.0, 1.0, op0=mybir.AluOpType.max, op1=mybir.AluOpType.min
                )
                prod = ep.tile([128, HIDDEN], F32, tag="prod")
                nc.vector.tensor_tensor(
                    prod, cl, v_t[:, p, :], op=mybir.AluOpType.mult
                )
                if p == 0:
                    acc0 = ep.tile([128, 1], F32, tag="acc0")
                    nc.vector.tensor_reduce(
                        acc0, prod, axis=mybir.AxisListType.X, op=mybir.AluOpType.add
                    )
                else:
                    acc1 = ep.tile([128, 1], F32, tag="acc1")
                    nc.vector.tensor_reduce(
                        acc1, prod, axis=mybir.AxisListType.X, op=mybir.AluOpType.add
                    )
                    # out[:, t] = (acc0 + b2) + acc1
                    nc.vector.scalar_tensor_tensor(
                        outst[:, t : t + 1],
                        acc0,
                        b2_t,
                        acc1,
                        op0=mybir.AluOpType.add,
                        op1=mybir.AluOpType.add,
                    )
            nc.sync.dma_start(out=outd[:, :], in_=outst)

    nc.compile()
    return nc


def _prep(x1, x2, l1_weights, l1_biases, l2_weight, l2_bias):
    """Host-side: tables, per-core index lists + selector matrices."""
    wt = np.ascontiguousarray(
        l1_weights.astype(np.float32).transpose(0, 2, 1)
    )  # [2, I, H]
    tabs = {}
    for p in range(2):
        for h in range(2):
            tt = np.zeros((HALF + 1, HIDDEN), dtype=NP_BF16)
            tt[:HALF] = wt[p, h * HALF : (h + 1) * HALF].astype(NP_BF16)
            tabs[f"t{p}{h}"] = tt

    bias_full = np.ascontiguousarray(
        np.broadcast_to(
            l1_biases.astype(np.float32)[None, :, :], (128, 2, HIDDEN)
        )
    )
    v_full = np.ascontiguousarray(
        np.broadcast_to(
            l2_weight.astype(np.float32).reshape(1, 2, HIDDEN), (128, 2, HIDDEN)
        )
    )
    b2_full = np.full((128, 1), float(np.asarray(l2_bias).reshape(-1)[0]), np.float32)

    xs = [np.asarray(x1), np.asarray(x2)]
    # (core, tile, persp, half) -> (rows, feats) of active entries
    active = []
    max_n = 0
    for c in range(N_CORES):
        per_core = []
        for i in range(2 * N_TILES):
            t, p = i // 2, i % 2
            blk = xs[p][c * B_CORE + t * 128 : c * B_CORE + (t + 1) * 128]
            r_all, f_all = np.nonzero(blk)
            for h in range(2):
                sel = (f_all >= h * HALF) & (f_all < (h + 1) * HALF)
                r, f = r_all[sel], f_all[sel] - h * HALF
                per_core.append((r.astype(np.int32), f.astype(np.int32)))
                max_n = max(max_n, len(r))
        active.append(per_core)

    nh = max(2048, -(-max_n // 128) * 128)
    nhc = nh // 16
    ncol = nh // 128
    cc = 2 * ncol

    in_maps = []
    for c in range(N_CORES):
        idx_arr = np.empty((16, 128, nhc), np.int16)
        smat = np.zeros((8, 128, cc * 128), NP_F8)
        for i in range(2 * N_TILES):
            for h in range(2):
                r, f = active[c][i * 2 + h]
                n = len(r)
                idxv = np.full(nh, HALF, np.int16)
                idxv[:n] = f
                idx_arr[i * 2 + h] = np.tile(idxv.reshape(nhc, 16).T, (8, 1))
                j = np.arange(n)
                smat[i, j % 128, (j // 128 + h * ncol) * 128 + r] = 1.0
        in_map = dict(tabs)
        in_map.update(
            idx=idx_arr,
            smat=smat,
            bias=bias_full,
            v=v_full,
            b2=b2_full,
        )
        in_maps.append(in_map)
    return nh, in_maps


def _run(x1, x2, l1_weights, l1_biases, l2_weight, l2_bias, trace=False):
    nh, in_maps = _prep(x1, x2, l1_weights, l1_biases, l2_weight, l2_bias)
    if nh not in _NC_CACHE:
        _NC_CACHE[nh] = _build(nh)
    nc = _NC_CACHE[nh]

    res = run_bass_kernel_spmd(
        nc, in_maps, core_ids=list(range(N_CORES)), trace=trace
    )
    out = np.concatenate(
        [
            np.ascontiguousarray(res.results[c]["out"].T).reshape(B_CORE)
            for c in range(N_CORES)
        ]
    )
    return out.astype(np.float32), res


def kernel(**inputs):
    out, _ = _run(**inputs)
    return out


def kernel_profiled(**inputs):
    _, res = _run(**inputs, trace=True)
    return res


# revision 9
# speedup vs baseline: 7.8895x; 1.0287x over previous
"""NNUE forward kernel for Trainium2, 8-core SPMD, batch-sharded,
sparsity-exploiting (embedding-gather formulation).

Reference computation (B=4096, I=40960, H=256):
    h_p = clip(x_p @ W_p.T + b_p, 0, 1)   for p in {1,2}
    out = concat(h1, h2) @ v + b2         -> (B,)

x_p rows are sparse binary (~30 active features of 40960), so
x_p @ W_p.T is an embedding-sum: h[b] = sum_{active f} W_p.T[f, :].

Per core (512 batch rows): for each 128-row tile and perspective,
DMA-gather the active rows of the bf16 table W_p.T (split into two
20480-row halves so indices fit int16), then reduce the gathered slots
into per-row sums on the PE with a per-tile 0/1 selector matrix S
(fp8): h_tile[128, 256] = S.T @ gathered. Epilogue (bias, clip, dot
with v, + b2) runs on the Vector engine. No collectives (pure data
parallel; batch-sharded).
"""

import numpy as np
import ml_dtypes

import concourse.bass as bass
import concourse.mybir as mybir
from concourse import bacc
from concourse.tile import TileContext
from concourse.bass_utils import run_bass_kernel_spmd

BATCH = 4096
INPUT_SIZE = 40960
HIDDEN = 256
N_CORES = 8
B_CORE = BATCH // N_CORES  # 512
N_TILES = B_CORE // 128  # 4
HALF = INPUT_SIZE // 2  # 20480 rows per table half (int16 index range)

BF16 = mybir.dt.bfloat16
F32 = mybir.dt.float32
F8 = mybir.dt.float8e4
I16 = mybir.dt.int16

NP_BF16 = ml_dtypes.bfloat16
NP_F8 = mybir.dt.np(F8)

_NC_CACHE = {}


def _build(nh):
    """nh: padded gather count per (128-row tile, perspective, half)."""
    nhc = nh // 16  # idx columns (16-partition wrap)
    ncol = nh // 128  # gather output columns per half
    cc = 2 * ncol  # selector columns per (tile, persp)

    nc = bacc.Bacc(
        "TRN2", target_bir_lowering=False, debug=False, num_swdge_queues=4
    )

    tbl = [
        [
            nc.dram_tensor(f"t{p}{h}", [HALF + 1, HIDDEN], BF16, kind="ExternalInput")
            for h in range(2)
        ]
        for p in range(2)
    ]
    idxd = nc.dram_tensor("idx", [16, 128, nhc], I16, kind="ExternalInput")
    smatd = nc.dram_tensor("smat", [8, 128, cc * 128], F8, kind="ExternalInput")
    biasd = nc.dram_tensor("bias", [128, 2, HIDDEN], F32, kind="ExternalInput")
    vd = nc.dram_tensor("v", [128, 2, HIDDEN], F32, kind="ExternalInput")
    b2d = nc.dram_tensor("b2", [128, 1], F32, kind="ExternalInput")
    outd = nc.dram_tensor("out", [128, N_TILES], F32, kind="ExternalOutput")

    with TileContext(nc) as tc:
        with (
            tc.tile_pool(name="consts", bufs=1) as consts,
            tc.tile_pool(name="sp", bufs=2) as sp,
            tc.tile_pool(name="g0p", bufs=2) as g0p,
            tc.tile_pool(name="g1p", bufs=2) as g1p,
            tc.tile_pool(name="psum", bufs=2, space="PSUM") as pp,
            tc.tile_pool(name="ep", bufs=2) as ep,
        ):
            idxt = consts.tile([128, 16, nhc], I16, tag="idx")
            for g in range(16):
                nc.sync.dma_start(out=idxt[:, g, :], in_=idxd[g, :, :])
            bias_t = consts.tile([128, 2, HIDDEN], F32, tag="bias")
            nc.sync.dma_start(out=bias_t, in_=biasd[:, :, :])
            v_t = consts.tile([128, 2, HIDDEN], F32, tag="v")
            nc.sync.dma_start(out=v_t, in_=vd[:, :, :])
            b2_t = consts.tile([128, 1], F32, tag="b2")
            nc.sync.dma_start(out=b2_t, in_=b2d[:, :])
            outst = consts.tile([128, N_TILES], F32, tag="outst")

            acc0 = None
            for i in range(2 * N_TILES):
                t, p = i // 2, i % 2
                s_t = sp.tile([128, cc * 128], F8, tag="smat")
                nc.sync.dma_start(out=s_t, in_=smatd[i, :, :])
                g0 = g0p.tile([128, ncol, HIDDEN], BF16, tag="g0")
                g1 = g1p.tile([128, ncol, HIDDEN], BF16, tag="g1")
                # HW SWDGE limit: >1024 idxs in one dma_gather wedges the
                # exec unit — chunk to <=1024 (8 output columns) per call.
                q = 0
                for h, gt in ((0, g0), (1, g1)):
                    for k0 in range(0, nh, 1024):
                        k1 = min(k0 + 1024, nh)
                        nc.gpsimd.dma_gather(
                            gt[:, k0 // 128 : k1 // 128, :],
                            tbl[p][h][:, :],
                            idxt[:, 2 * i + h, k0 // 16 : k1 // 16],
                            k1 - k0,
                            k1 - k0,
                            HIDDEN,
                            queue_num=q % 4,
                        )
                        q += 1
                psum = pp.tile([128, HIDDEN], F32, tag="psum")
                for j in range(cc):
                    g = g0 if j < ncol else g1
                    nc.tensor.matmul(
                        psum,
                        lhsT=s_t[:, j * 128 : (j + 1) * 128],
                        rhs=g[:, j % ncol, :],
                        start=(j == 0),
                        stop=(j == cc - 1),
                    )
                cl = ep.tile([128, HIDDEN], F32, tag="cl")
                nc.vector.tensor_tensor(cl, psum, bias_t[:, p, :], op=mybir.AluOpType.add)
                cl2 = ep.tile([128, HIDDEN], F32, tag="cl2")
                nc.vector.tensor_scalar(
                    cl2, cl, 0.0, 1.0, op0=mybir.AluOpType.max, op1=mybir.AluOpType.min
                )
                prod = ep.tile([128, HIDDEN], F32, tag="prod")
                nc.vector.tensor_tensor(
                    prod, cl2, v_t[:, p, :], op=mybir.AluOpType.mult
                )
                if p == 0:
                    acc0 = ep.tile([128, 1], F32, tag="acc0")
                    nc.vector.tensor_reduce(
                        acc0, prod, axis=mybir.AxisListType.X, op=mybir.AluOpType.add
                    )
                else:
                    acc1 = ep.tile([128, 1], F32, tag="acc1")
                    nc.vector.tensor_reduce(
                        acc1, prod, axis=mybir.AxisListType.X, op=mybir.AluOpType.add
                    )
                    # out[:, t] = (acc0 + b2) + acc1
                    nc.vector.scalar_tensor_tensor(
                        outst[:, t : t + 1],
                        acc0,
                        b2_t,
                        acc1,
                        op0=mybir.AluOpType.add,
                        op1=mybir.AluOpType.add,
                    )
            nc.sync.dma_start(out=outd[:, :], in_=outst)

    nc.compile()
    return nc


def _prep(x1, x2, l1_weights, l1_biases, l2_weight, l2_bias):
    """Host-side: tables, per-core index lists + selector matrices."""
    wt = np.ascontiguousarray(
        l1_weights.astype(np.float32).transpose(0, 2, 1)
    )  # [2, I, H]
    tabs = {}
    for p in range(2):
        for h in range(2):
            tt = np.zeros((HALF + 1, HIDDEN), dtype=NP_BF16)
            tt[:HALF] = wt[p, h * HALF : (h + 1) * HALF].astype(NP_BF16)
            tabs[f"t{p}{h}"] = tt

    bias_full = np.ascontiguousarray(
        np.broadcast_to(
            l1_biases.astype(np.float32)[None, :, :], (128, 2, HIDDEN)
        )
    )
    v_full = np.ascontiguousarray(
        np.broadcast_to(
            l2_weight.astype(np.float32).reshape(1, 2, HIDDEN), (128, 2, HIDDEN)
        )
    )
    b2_full = np.full((128, 1), float(np.asarray(l2_bias).reshape(-1)[0]), np.float32)

    xs = [np.asarray(x1), np.asarray(x2)]
    # (core, tile, persp, half) -> (rows, feats) of active entries
    active = []
    max_n = 0
    for c in range(N_CORES):
        per_core = []
        for i in range(2 * N_TILES):
            t, p = i // 2, i % 2
            blk = xs[p][c * B_CORE + t * 128 : c * B_CORE + (t + 1) * 128]
            r_all, f_all = np.nonzero(blk)
            for h in range(2):
                sel = (f_all >= h * HALF) & (f_all < (h + 1) * HALF)
                r, f = r_all[sel], f_all[sel] - h * HALF
                per_core.append((r.astype(np.int32), f.astype(np.int32)))
                max_n = max(max_n, len(r))
        active.append(per_core)

    nh = max(2048, -(-max_n // 128) * 128)
    nhc = nh // 16
    ncol = nh // 128
    cc = 2 * ncol

    in_maps = []
    for c in range(N_CORES):
        idx_arr = np.empty((16, 128, nhc), np.int16)
        smat = np.zeros((8, 128, cc * 128), NP_F8)
        for i in range(2 * N_TILES):
            for h in range(2):
                r, f = active[c][i * 2 + h]
                n = len(r)
                idxv = np.full(nh, HALF, np.int16)
                idxv[:n] = f
                idx_arr[i * 2 + h] = np.tile(idxv.reshape(nhc, 16).T, (8, 1))
                j = np.arange(n)
                smat[i, j % 128, (j // 128 + h * ncol) * 128 + r] = 1.0
        in_map = dict(tabs)
        in_map.update(
            idx=idx_arr,
            smat=smat,
            bias=bias_full,
            v=v_full,
            b2=b2_full,
        )
        in_maps.append(in_map)
    return nh, in_maps


def _run(x1, x2, l1_weights, l1_biases, l2_weight, l2_bias, trace=False):
    nh, in_maps = _prep(x1, x2, l1_weights, l1_biases, l2_weight, l2_bias)
    if nh not in _NC_CACHE:
        _NC_CACHE[nh] = _build(nh)
    nc = _NC_CACHE[nh]

    res = run_bass_kernel_spmd(
        nc, in_maps, core_ids=list(range(N_CORES)), trace=trace
    )
    out = np.concatenate(
        [
            np.ascontiguousarray(res.results[c]["out"].T).reshape(B_CORE)
            for c in range(N_CORES)
        ]
    )
    return out.astype(np.float32), res


def kernel(**inputs):
    out, _ = _run(**inputs)
    return out


def kernel_profiled(**inputs):
    _, res = _run(**inputs, trace=True)
    return res
